# revision 10
# baseline (speedup 1.0000x reference)
"""Trainium2 Bass kernel for nn_Net_76330158785143 (dense_cnn) — optimized.

Pipeline per sample: per-sample 11x11 autocorrelation of channel 2 ->
conv5x5(1->32) relu -> maxpool2 -> conv5x5(32->64) relu -> maxpool2 ->
conv3x3(64->10) relu -> GAP -> log_softmax.

Sharding: pure data parallel, batch 8192 -> 1024 per core across 8 cores.

Per-core structure (software-pipelined over 128-sample b-tiles, depth 3):
  iter t:  dup DMAs(t-2) | corr(t) interleaved with conv1(t-1) | late(t-2)
           | prep(t+1): x load, pad+cast, template, diag-bank generation
  late = conv2 (7 matmuls/cc via pre-shifted dup copies, K=128)
         + pool2 + conv3 (6 matmuls, K=128 pairs) + GAP
  final: transpose + log_softmax over all tiles (batched to avoid
         activation-table swaps), single output DMA.
"""

import sys

sys.path.insert(0, "/opt/trn_rl_repo")

import numpy as np

import concourse.bacc as bacc
import concourse.mybir as mybir
from concourse.ap import AP
from concourse.tile import TileContext
from concourse.bass_utils import run_bass_kernel_spmd

F32 = mybir.dt.float32
BF16 = mybir.dt.bfloat16
ALU = mybir.AluOpType
ACTF = mybir.ActivationFunctionType
AXIS = mybir.AxisListType

N_CORES = 8
B_FULL = 8192
B_CORE = B_FULL // N_CORES


def _build(nc, b_core):
    n_bt = b_core // 128  # 128-sample tiles

    x_d = nc.dram_tensor("x", [b_core, 3, 28, 28], F32, kind="ExternalInput")
    identc_d = nc.dram_tensor("identc", [128, 128], BF16, kind="ExternalInput")
    ident10_d = nc.dram_tensor("ident10", [16, 16], F32, kind="ExternalInput")
    w1p_d = nc.dram_tensor("w1p", [128, 128], BF16, kind="ExternalInput")
    w2p_d = nc.dram_tensor("w2p", [128, 448], BF16, kind="ExternalInput")
    w3p_d = nc.dram_tensor("w3p", [128, 64], BF16, kind="ExternalInput")
    b1p_d = nc.dram_tensor("b1p", [128, 1], F32, kind="ExternalInput")
    b2p_d = nc.dram_tensor("b2p", [64, 1], F32, kind="ExternalInput")
    b3p_d = nc.dram_tensor("b3p", [16, 1], F32, kind="ExternalInput")
    out_d = nc.dram_tensor("out", [b_core, 10], F32, kind="ExternalOutput")

    from contextlib import ExitStack

    with TileContext(nc) as tc, ExitStack() as es:
        def _pool(name, bufs, space=None):
            kw = {"space": space} if space else {}
            return es.enter_context(tc.tile_pool(name=name, bufs=bufs, **kw))

        cpool = _pool("const", 1)
        xfpool = _pool("xf", 1)
        imgpool = _pool("img", 2)
        tmplpool = _pool("tmpl", 1)
        dgpool = _pool("dg", 1)
        corrpool = _pool("corr", 1)
        accpool = _pool("acc", 1)
        s25pool = _pool("s25", 3)
        pxpool = _pool("px", 2)
        a1pool = _pool("a1", 1)
        duppool = _pool("dup", 2)
        qxpool = _pool("qx", 2)
        l3pool = _pool("l3", 2)
        lgbpool = _pool("lgb", 1)
        smpool = _pool("sm", 2)
        dscrpool = _pool("dscr", 2, "DRAM")
        pcorr = _pool("pcorr", 1, "PSUM")
        pc1 = _pool("pc1", 3, "PSUM")
        pc2 = _pool("pc2", 2, "PSUM")
        pc3 = _pool("pc3", 1, "PSUM")

        if True:
            def _load_const(name, dram, shape, dtype):
                t = cpool.tile(shape, dtype, name=name + "_sb")
                f = int(np.prod(shape[1:]))
                nc.sync.dma_start(
                    out=AP(t.tensor, 0, [[f, shape[0]], [1, f]]),
                    in_=AP(dram, 0, [[f, shape[0]], [1, f]]),
                )
                return t

            identc = _load_const("identc", identc_d, [128, 128], BF16)
            ident10 = _load_const("ident10", ident10_d, [16, 16], F32)
            w1p = _load_const("w1p", w1p_d, [128, 128], BF16)
            w2p = _load_const("w2p", w2p_d, [128, 448], BF16)
            w3p = _load_const("w3p", w3p_d, [128, 64], BF16)
            b1p = _load_const("b1p", b1p_d, [128, 1], F32)
            b2p = _load_const("b2p", b2p_d, [64, 1], F32)
            b3p = _load_const("b3p", b3p_d, [16, 1], F32)
            zpad = cpool.tile([32, 64], BF16, name="zpad_sb")
            nc.gpsimd.memset(zpad[:, :], 0.0)

            # logits for the whole core batch, softmaxed at the end
            logitsb = lgbpool.tile([32, n_bt * 128], F32)
            img_bufs = [
                imgpool.tile([128, 38 * 38], BF16, name=f"imgbuf{i}")
                for i in range(2)
            ]

            # rotating state, keyed by tile index
            st = {}
            ND = 17  # corr taps offloaded to DVE
            PE_TAPS = 121 - ND

            def prep(t):
                """x load + pad/cast + template + diag bank for tile t."""
                s = st.setdefault(t, {})
                xf = xfpool.tile([128, 784], F32, name="xf")
                nc.sync.dma_start(
                    out=AP(xf.tensor, 0, [[784, 128], [1, 784]]),
                    in_=AP(x_d, t * 128 * 2352 + 2 * 784, [[2352, 128], [1, 784]]),
                )
                img = img_bufs[t % 2]
                # cast f32 -> bf16 into padded interior (borders pre-zeroed)
                nc.scalar.activation(
                    AP(img.tensor, 5 * 38 + 5, [[1444, 128], [38, 28], [1, 28]]),
                    xf[:, :],
                    ACTF.Copy,
                )
                tmpl = tmplpool.tile([128, 121], BF16, name="tmpl")
                nc.vector.tensor_copy(
                    out=tmpl[:, :],
                    in_=AP(img.tensor, 13 * 38 + 13, [[1444, 128], [38, 11], [1, 11]]),
                )
                tmplf = tmplpool.tile([128, 121], F32, name="tmplf")
                nc.vector.tensor_copy(out=tmplf[:, :], in_=tmpl[:, :])
                # diag bank: dg[s, tap*128 + m] = tmpl[s, tap] * (s == m)
                dg = dgpool.tile([128, PE_TAPS * 128], BF16, name="dg")
                for c0 in range(0, PE_TAPS, 16):
                    n = min(16, PE_TAPS - c0)
                    nc.gpsimd.tensor_tensor(
                        out=AP(dg.tensor, c0 * 128, [[PE_TAPS * 128, 128], [1, n * 128]]),
                        in0=AP(tmpl.tensor, c0, [[121, 128], [1, n], [0, 128]]),
                        in1=AP(identc.tensor, 0, [[128, 128], [0, n], [1, 128]]),
                        op=ALU.mult,
                    )
                s["img"] = img
                s["dg"] = dg
                s["tmpl"] = tmpl
                s["tmplf"] = tmplf
                s["accD"] = accpool.tile([128, 784], F32, tag="accD", name="accD")

            def emit_corr_tap(t, tap):
                s = st[t]
                img, dg = s["img"], s["dg"]
                u, v = tap // 11, tap % 11
                lhsT = AP(dg.tensor, tap * 128, [[PE_TAPS * 128, 128], [1, 128]])
                nc.tensor.matmul(
                    s["ps_a"][:, :],
                    lhsT,
                    AP(img.tensor, u * 38 + v, [[1444, 128], [38, 14], [1, 28]]),
                    start=(tap == 0),
                    stop=(tap == PE_TAPS - 1),
                )
                nc.tensor.matmul(
                    s["ps_b"][:, :],
                    lhsT,
                    AP(img.tensor, (u + 14) * 38 + v, [[1444, 128], [38, 14], [1, 28]]),
                    start=(tap == 0),
                    stop=(tap == PE_TAPS - 1),
                )

            def emit_corr_tap_vec(t, tap, eng, acc, first):
                s = st[t]
                img, tmpl = s["img"], s["tmplf"]
                u, v = tap // 11, tap % 11
                src_ap = AP(img.tensor, u * 38 + v, [[1444, 128], [38, 28], [1, 28]])
                if first:
                    eng.tensor_scalar_mul(acc[:, :], src_ap, tmpl[:, tap : tap + 1])
                else:
                    eng.scalar_tensor_tensor(
                        out=acc[:, :],
                        in0=src_ap,
                        scalar=tmpl[:, tap : tap + 1],
                        in1=acc[:, :],
                        op0=ALU.mult,
                        op1=ALU.add,
                    )

            def corr_finish(t):
                """Drain corr psum, roundtrip via DRAM into s25 layout."""
                s = st[t]
                corr = corrpool.tile([128, 784], BF16, name="corr")
                accD = s["accD"]
                nc.vector.tensor_add(corr[:, 0:392], s["ps_a"][:, :], accD[:, 0:392])
                nc.vector.tensor_add(
                    corr[:, 392:784], s["ps_b"][:, :], accD[:, 392:784]
                )
                # corr_d25[tap, s, j] = corr[s, (tap//5)*28 + (tap%5) + j]
                corr_d = dscrpool.tile(
                    [25, 128 * 668], BF16, tag="corr_d", name="corr_d"
                )
                for ky in range(5):
                    nc.sync.dma_start(
                        out=AP(
                            corr_d.tensor,
                            ky * 5 * (128 * 668),
                            [[668, 128], [128 * 668, 5], [1, 668]],
                        ),
                        in_=AP(corr.tensor, ky * 28, [[784, 128], [1, 5], [1, 668]]),
                    )
                # s25[p = g*25 + tap, slot*704 + j], slot = (sub%2)*8 + q
                s25s = []
                P = 16 * 672
                for half in range(2):
                    s25 = s25pool.tile([128, P], BF16, name="s25")
                    for g in range(4):
                        eng = nc.sync if g % 2 == 0 else nc.scalar
                        eng.dma_start(
                            out=AP(
                                s25.tensor,
                                g * 25 * P,
                                [[P, 25], [672, 16], [1, 668]],
                            ),
                            in_=AP(
                                corr_d.tensor,
                                (half * 64 + g) * 668,
                                [[128 * 668, 25], [4 * 668, 16], [1, 668]],
                            ),
                        )
                    s25s.append(s25)
                s["s25"] = s25s

            def conv1_mm(t, sub, q, h):
                """One conv1 matmul: 4 samples (q-group), half-image h."""
                s = st[t]
                ps1 = pc1.tile([128, 288], F32, tag="ps1", name="ps1")
                nc.tensor.matmul(
                    ps1[:, :],
                    AP(w1p.tensor, 0, [[128, 100], [1, 128]]),
                    AP(
                        s["s25"][sub // 2].tensor,
                        ((sub % 2) * 8 + q) * 672 + h * 336,
                        [[16 * 672, 100], [28, 12], [1, 24]],
                    ),
                    start=True,
                    stop=True,
                )
                return ps1

            def conv1_drain(t, sub, q, h, ps1, use_act):
                s = st[t]
                px = s["px"]
                o = AP(px.tensor, (q * 2 + h) * 288, [[4608, 128], [1, 288]])
                if use_act:
                    nc.scalar.activation(o, ps1[:, :], ACTF.Copy)
                else:
                    nc.vector.tensor_copy(out=o, in_=ps1[:, :])

            def conv1_sub_begin(t, sub):
                s = st[t]
                s["px"] = pxpool.tile([128, 16 * 288], BF16, name="px")

            def conv1_sub_end(t, sub):
                s = st[t]
                px = s["px"]
                a1 = s["a1"]
                # fused 2x2 maxpool: reduce the (dy, dx) window dims
                nc.vector.tensor_reduce(
                    out=AP(a1.tensor, sub * 1152, [[4608, 128], [1, 1152]]),
                    in_=AP(
                        px.tensor,
                        0,
                        [[4608, 128], [48, 96], [2, 12], [24, 2], [1, 2]],
                    ),
                    axis=AXIS.XY,
                    op=ALU.max,
                )
                # bias + relu in place
                nc.scalar.activation(
                    AP(a1.tensor, sub * 1152, [[4608, 128], [1, 1152]]),
                    AP(a1.tensor, sub * 1152, [[4608, 128], [1, 1152]]),
                    ACTF.Relu,
                    bias=b1p[:, 0:1],
                )

            def conv1_tile_begin(t):
                st[t]["a1"] = a1pool.tile([128, 32 * 144], BF16, name="a1")

            def conv1_tile_end(t):
                """Write a1 to DRAM in ci-major layout (one DMA per g)."""
                s = st[t]
                a1 = s["a1"]
                a1_d = dscrpool.tile(
                    [32, 128 * 144 + 64], BF16, tag="a1_d", name="a1_d"
                )
                for g in range(4):
                    nc.gpsimd.dma_start(
                        out=AP(
                            a1_d.tensor,
                            g * 144,
                            [[128 * 144 + 64, 32], [576, 32], [1, 144]],
                        ),
                        in_=AP(a1.tensor, g * 32 * 4608, [[4608, 32], [1, 4608]]),
                    )
                nc.gpsimd.dma_start(
                    out=AP(a1_d.tensor, 128 * 144, [[128 * 144 + 64, 32], [1, 64]]),
                    in_=zpad[:, :],
                )
                s["a1_d"] = a1_d

            def dup_dma(t):
                """Build pre-shifted conv2 input copies for all 4 subs."""
                s = st[t]
                a1_d = s["a1_d"]
                dups = []
                for sub in range(4):
                    dupa = duppool.tile([128, 4608], BF16, name="dupa")
                    dupb = duppool.tile([128, 4608], BF16, name="dupb")
                    # copy r at partitions 32r.. shifted r columns (flat)
                    (nc.sync if sub % 2 == 0 else nc.scalar).dma_start(
                        out=AP(dupa.tensor, 0, [[4608, 128], [1, 4608]]),
                        in_=AP(
                            a1_d.tensor,
                            sub * 4608,
                            [[1, 4], [128 * 144 + 64, 32], [1, 4608]],
                        ),
                    )
                    # copy r shifted r rows + 4 columns (flat 12r + 4)
                    (nc.scalar if sub % 2 == 0 else nc.sync).dma_start(
                        out=AP(dupb.tensor, 0, [[4608, 128], [1, 4608]]),
                        in_=AP(
                            a1_d.tensor,
                            sub * 4608 + 4,
                            [[12, 4], [128 * 144 + 64, 32], [1, 4608]],
                        ),
                    )
                    dups.append((dupa, dupb))
                s["dups"] = dups

            def conv2_cc(t, sub, cc):
                s = st[t]
                dupa, dupb = s["dups"][sub]
                ps2 = pc2.tile([64, 512], F32, tag="ps2", name="ps2")
                for dy in range(5):
                    nc.tensor.matmul(
                        ps2[:, :],
                        AP(w2p.tensor, dy * 64, [[448, 128], [1, 64]]),
                        AP(
                            dupa.tensor,
                            cc * 8 * 144 + dy * 12,
                            [[4608, 128], [144, 8], [12, 8], [1, 8]],
                        ),
                        start=(dy == 0),
                        stop=False,
                    )
                nc.tensor.matmul(
                    ps2[:, :],
                    AP(w2p.tensor, 5 * 64, [[448, 128], [1, 64]]),
                    AP(
                        dupb.tensor,
                        cc * 8 * 144,
                        [[4608, 128], [144, 8], [12, 8], [1, 8]],
                    ),
                    start=False,
                    stop=False,
                )
                nc.tensor.matmul(
                    ps2[:, :],
                    AP(w2p.tensor, 6 * 64, [[448, 32], [1, 64]]),
                    AP(
                        dupa.tensor,
                        cc * 8 * 144 + 4 * 12 + 4,
                        [[4608, 32], [144, 8], [12, 8], [1, 8]],
                    ),
                    start=False,
                    stop=True,
                    tile_position=(0, 0),
                )
                return ps2

            def late_conv2_begin(t, sub):
                s = st[t]
                s.setdefault("qxs", {})[sub] = qxpool.tile([64, 2048], BF16, name="qx")

            def late_conv2_cc(t, sub, cc):
                s = st[t]
                ps2 = conv2_cc(t, sub, cc)
                qx = s["qxs"][sub]
                o = AP(qx.tensor, cc * 512, [[2048, 64], [1, 512]])
                if cc % 2 == 0:
                    nc.scalar.activation(o, ps2[:, :], ACTF.Copy)
                else:
                    nc.vector.tensor_copy(out=o, in_=ps2[:, :])

            def late_conv2_finish(t, sub):
                s = st[t]
                qx = s["qxs"].pop(sub)
                qy = qxpool.tile([64, 512], BF16, name="qy")
                # fused 2x2 maxpool: qx layout (cc, s8, y8, x8)
                nc.vector.tensor_reduce(
                    out=qy[:, :],
                    in_=AP(
                        qx.tensor,
                        0,
                        [[2048, 64], [16, 128], [2, 4], [8, 2], [1, 2]],
                    ),
                    axis=AXIS.XY,
                    op=ALU.max,
                )
                l3 = l3pool.tile([128, 512], BF16, name="l3")
                nc.scalar.activation(
                    l3[0:64, :], qy[:, :], ACTF.Relu, bias=b2p[:, 0:1]
                )
                # rows 64..127 = shift-1-pixel relu copy (for conv3 tap pairs)
                nc.scalar.activation(
                    AP(l3.tensor, 64 * 512, [[512, 64], [1, 511]]),
                    AP(qy.tensor, 1, [[512, 64], [1, 511]]),
                    ACTF.Relu,
                    bias=b2p[:, 0:1],
                )
                s.setdefault("l3s", {})[sub] = l3

            def late_conv3(t, sub):
                s = st[t]
                l3 = s["l3s"].pop(sub)
                # conv3: 3 K=64 singles first (rows 0..63 only), then 3 K=128 pairs
                ps3 = pc3.tile([16, 128], F32, tag="ps3", name="ps3")
                for k in range(3):
                    nc.tensor.matmul(
                        ps3[0:10, :],
                        AP(w3p.tensor, (3 + k) * 10, [[64, 64], [1, 10]]),
                        AP(
                            l3.tensor,
                            k * 4 + 2,
                            [[512, 64], [16, 32], [4, 2], [1, 2]],
                        ),
                        start=(k == 0),
                        stop=False,
                        tile_position=(0, 0),
                    )
                for dy in range(3):
                    nc.tensor.matmul(
                        ps3[0:10, :],
                        AP(w3p.tensor, dy * 10, [[64, 128], [1, 10]]),
                        AP(l3.tensor, dy * 4, [[512, 128], [16, 32], [4, 2], [1, 2]]),
                        start=False,
                        stop=(dy == 2),
                    )
                r3 = smpool.tile([16, 128], F32, name="r3")
                nc.scalar.activation(
                    r3[0:10, :], ps3[0:10, :], ACTF.Relu, bias=b3p[0:10, 0:1]
                )
                # GAP: sum over the 2x2 window (0.25 folded into w3/b3)
                nc.vector.tensor_reduce(
                    out=logitsb[0:10, (t * 4 + sub) * 32 : (t * 4 + sub) * 32 + 32],
                    in_=AP(r3.tensor, 0, [[128, 10], [4, 32], [1, 4]]),
                    axis=AXIS.X,
                    op=ALU.add,
                )

            def late(t):
                for sub in range(4):
                    late_conv2_begin(t, sub)
                    for cc in range(4):
                        late_conv2_cc(t, sub, cc)
                    late_conv2_finish(t, sub)
                    if sub >= 1:
                        late_conv3(t, sub - 1)
                late_conv3(t, 3)

            # ---------------- main pipeline ----------------
            for i in range(2):
                nc.gpsimd.memset(img_bufs[i][:, :], 0.0)
            # zero logits (incl pad rows 10..31 read by the final transpose)
            nc.gpsimd.memset(logitsb[:, :], 0.0)
            prep(0)
            for it in range(n_bt + 2):
                t_late = it - 2
                t_c1 = it - 1
                if 0 <= t_late < n_bt:
                    dup_dma(t_late)
                do_corr = it < n_bt
                do_c1 = 0 <= t_c1 < n_bt
                do_late = 0 <= t_late < n_bt
                if do_corr:
                    s = st[it]
                    s["ps_a"] = pcorr.tile([128, 392], F32, tag="cpa", name="cpa")
                    s["ps_b"] = pcorr.tile([128, 392], F32, tag="cpb", name="cpb")
                if do_c1:
                    conv1_tile_begin(t_c1)
                # job list: closures, emitted interleaved into the corr stream
                jobs = []
                if do_c1:
                    for sub in range(4):
                        jobs.append((conv1_sub_begin, (t_c1, sub)))
                        for q in range(8):
                            for h in range(2):
                                jobs.append((None, (t_c1, sub, q, h)))
                        jobs.append((conv1_sub_end, (t_c1, sub)))
                if do_late:
                    for sub in range(4):
                        jobs.append((late_conv2_begin, (t_late, sub)))
                        for cc in range(4):
                            jobs.append((late_conv2_cc, (t_late, sub, cc)))
                        jobs.append((late_conv2_finish, (t_late, sub)))
                        if sub >= 1:
                            jobs.append((late_conv3, (t_late, sub - 1)))
                    jobs.append((late_conv3, (t_late, 3)))
                ji = 0

                def run_jobs(upto):
                    nonlocal ji
                    while ji < len(jobs) and ji < upto:
                        fn, args = jobs[ji]
                        if fn is None:
                            tt, sub, q, h = args
                            ps1 = conv1_mm(tt, sub, q, h)
                            conv1_drain(tt, sub, q, h, ps1, (q * 2 + h) % 4 < 1)
                        else:
                            fn(*args)
                        ji += 1

                if do_corr:
                    nj = len(jobs)
                    for tap in range(PE_TAPS):
                        emit_corr_tap(it, tap)
                        # offloaded taps, spread across the PE tap stream
                        for k in range(ND):
                            if k * PE_TAPS // ND == tap:
                                emit_corr_tap_vec(
                                    it, PE_TAPS + k, nc.vector, st[it]["accD"], k == 0
                                )
                        if tap < 24:
                            run_jobs(min((tap + 1) * nj // PE_TAPS, 24 * nj // PE_TAPS))
                        else:
                            run_jobs((tap + 1) * nj // PE_TAPS)
                    corr_finish(it)
                run_jobs(len(jobs))
                if do_c1:
                    conv1_tile_end(t_c1)
                if it + 1 < n_bt:
                    prep(it + 1)
                # free state of finished tiles
                if t_late >= 0:
                    st.pop(t_late, None)

            # ---------------- final: transpose + log_softmax ----------------
            for t in range(n_bt):
                psT = smpool.tile([128, 32], F32, name="psT")
                for b in range(4):
                    nc.vector.transpose(
                        out=AP(psT.tensor, b * 32 * 32, [[32, 32], [1, 32]]),
                        in_=AP(
                            logitsb.tensor,
                            t * 128 + b * 32,
                            [[n_bt * 128, 32], [1, 32]],
                        ),
                    )
                mx = smpool.tile([128, 1], F32, name="mx")
                nc.vector.tensor_reduce(
                    out=mx[:, :], in_=psT[:, 0:10], axis=AXIS.X, op=ALU.max
                )
                negmx = smpool.tile([128, 1], F32, name="negmx")
                nc.vector.tensor_scalar_mul(negmx[:, :], mx[:, :], -1.0)
                ex = smpool.tile([128, 16], F32, name="ex")
                nc.scalar.activation(
                    ex[:, 0:10], psT[:, 0:10], ACTF.Exp, bias=negmx[:, 0:1]
                )
                sm = smpool.tile([128, 1], F32, name="sm")
                nc.vector.tensor_reduce(
                    out=sm[:, :], in_=ex[:, 0:10], axis=AXIS.X, op=ALU.add
                )
                lsm = smpool.tile([128, 1], F32, name="lsm")
                nc.scalar.activation(lsm[:, :], sm[:, :], ACTF.Ln)
                off = smpool.tile([128, 1], F32, name="off")
                nc.vector.tensor_add(off[:, :], mx[:, :], lsm[:, :])
                outt = smpool.tile([128, 16], F32, name="outt")
                nc.vector.tensor_scalar(
                    outt[:, 0:10], psT[:, 0:10], off[:, 0:1], None, ALU.subtract
                )
                nc.sync.dma_start(
                    out=AP(out_d, t * 1280, [[10, 128], [1, 10]]),
                    in_=outt[:, 0:10],
                )
    return nc


_CACHE = {}


def _get_nc(b_core):
    if b_core not in _CACHE:
        nc = bacc.Bacc(
            "TRN2", target_bir_lowering=False, debug=False, num_devices=N_CORES
        )
        _build(nc, b_core)
        nc.compile()
        _CACHE[b_core] = nc
    return _CACHE[b_core]


def _prep_inputs(inputs):
    import ml_dtypes

    bf16 = ml_dtypes.bfloat16
    w1 = np.asarray(inputs["w1"], dtype=np.float32).reshape(32, 5, 5)
    w2 = np.asarray(inputs["w2"], dtype=np.float32)  # [64, 32, 5, 5]
    w3 = np.asarray(inputs["w3"], dtype=np.float32)  # [10, 64, 3, 3]
    b1 = np.asarray(inputs["b1"], dtype=np.float32)
    b2 = np.asarray(inputs["b2"], dtype=np.float32)
    b3 = np.asarray(inputs["b3"], dtype=np.float32)

    # conv1: lhsT rows p = tap*4 + g, cols m = g*32 + co
    w1p = np.zeros((128, 128), dtype=bf16)
    for ky in range(5):
        for kx in range(5):
            tap = ky * 5 + kx
            for g in range(4):
                w1p[g * 25 + tap, g * 32 : g * 32 + 32] = w1[:, ky, kx].astype(bf16)

    # conv2: blocks dy=0..4: rows r*32+ci = w2[co, ci, dy, r]
    #        block 5: rows r*32+ci = w2[co, ci, r, 4]
    #        block 6: rows ci      = w2[co, ci, 4, 4]
    w2p = np.zeros((128, 448), dtype=bf16)
    for dy in range(5):
        for r in range(4):
            w2p[r * 32 : r * 32 + 32, dy * 64 : dy * 64 + 64] = (
                w2[:, :, dy, r].T.astype(bf16)
            )
    for r in range(4):
        w2p[r * 32 : r * 32 + 32, 320:384] = w2[:, :, r, 4].T.astype(bf16)
    w2p[0:32, 384:448] = w2[:, :, 4, 4].T.astype(bf16)

    # conv3 (0.25 GAP folded in): blocks dy=0..2 pairs (dy,0)/(dy,1),
    # blocks 3..5 singles (k,2)
    w3q = (0.25 * w3).astype(np.float32)
    w3p = np.zeros((128, 64), dtype=bf16)
    for dy in range(3):
        w3p[0:64, dy * 10 : dy * 10 + 10] = w3q[:, :, dy, 0].T.astype(bf16)
        w3p[64:128, dy * 10 : dy * 10 + 10] = w3q[:, :, dy, 1].T.astype(bf16)
    for k in range(3):
        w3p[0:64, (3 + k) * 10 : (3 + k) * 10 + 10] = w3q[:, :, k, 2].T.astype(bf16)

    b1p = np.tile(b1, 4).reshape(128, 1)
    b2p = b2.reshape(64, 1)
    b3p = np.pad(0.25 * b3, (0, 6)).reshape(16, 1)
    identc = np.eye(128, dtype=bf16)
    ident10 = np.eye(16, dtype=np.float32)
    return dict(
        identc=identc,
        ident10=ident10,
        w1p=w1p,
        w2p=w2p,
        w3p=w3p,
        b1p=b1p,
        b2p=b2p,
        b3p=b3p,
    )


def _run(inputs, b_core=B_CORE, trace=False):
    x = np.ascontiguousarray(np.asarray(inputs["x"], dtype=np.float32))
    consts = _prep_inputs(inputs)
    nc = _get_nc(b_core)
    in_maps = [
        {"x": x[i * b_core : (i + 1) * b_core], **consts} for i in range(N_CORES)
    ]
    res = run_bass_kernel_spmd(nc, in_maps, core_ids=list(range(N_CORES)), trace=trace)
    out = np.concatenate([res.results[i]["out"] for i in range(N_CORES)], axis=0)
    return out.astype(np.float32), res


def kernel(**inputs) -> np.ndarray:
    out, _ = _run(inputs)
    return out
